# revision 3
# baseline (speedup 1.0000x reference)
"""CrissCross (axial) attention on 8 TRN2 NeuronCores — Bass/Tile kernel.

Contract: kernel(**inputs) takes the FULL inputs from setup_inputs() and
returns the FULL [16, 64, 128, 128] float32 output.

Sharding: pure data parallel — batch 16 split 2-per-core across 8 cores,
weights replicated (folded into bf16 constants on the host; gamma is folded
into the Wv constant since attention output is linear in v).

The device program (built in _bass_build) computes, per batch:
  q/k = 1x1 conv (PE matmul with bias via a ones-row trick)
  row/col energies -> exp -> (diag mask for the H axis) -> att = P @ v
  joint softmax denominator z from per-line partial sums emitted by the same
  AV matmuls (ones column in the vT stationary)
  out = (att_row + att_col) / z + x

See the in-file documentation of _bass_build for layout details.
"""

import numpy as np
import ml_dtypes

B, C, H, W = 16, 64, 128, 128
HW = H * W
N_CORES = 8
BL = B // N_CORES  # batches per core
NSTRIP = 16
CHUNK = 512

_bf16 = ml_dtypes.bfloat16


def _bass_build(n_batches):
    import concourse.bacc as bacc
    import concourse.mybir as mybir
    import concourse.tile as tile
    from contextlib import ExitStack

    f32 = mybir.dt.float32
    bf16 = mybir.dt.bfloat16
    AF = mybir.ActivationFunctionType
    ALU = mybir.AluOpType

    nc = bacc.Bacc("TRN2", target_bir_lowering=False, debug=False)

    x_d = nc.dram_tensor("x", [n_batches, C + 1, HW], bf16, kind="ExternalInput").ap()
    wqk_d = nc.dram_tensor("wqk", [65, 16], bf16, kind="ExternalInput").ap()
    wvt_d = nc.dram_tensor("wvt", [65, 65], bf16, kind="ExternalInput").ap()
    mask8_d = nc.dram_tensor("mask8", [128, 512], bf16, kind="ExternalInput").ap()
    identb_d = nc.dram_tensor("identb", [128, 128], bf16, kind="ExternalInput").ap()
    out_d = nc.dram_tensor("out", [n_batches, C, HW], f32, kind="ExternalOutput").ap()

    with tile.TileContext(nc) as tc, ExitStack() as ctx:
        const = ctx.enter_context(tc.tile_pool(name="const", bufs=1))
        # DMA-loaded consts get a full in-place DVE rewrite so PE consumers
        # see a DVE producer (keeps matmul wait lists short).
        wqk = const.tile([65, 16], bf16)
        nc.sync.dma_start(wqk, wqk_d)
        nc.vector.tensor_copy(wqk, wqk)
        wvt = const.tile([65, 65], bf16)
        nc.sync.dma_start(wvt, wvt_d)
        nc.vector.tensor_copy(wvt, wvt)
        identb = const.tile([128, 128], bf16)
        nc.sync.dma_start(identb, identb_d)
        nc.vector.tensor_copy(identb, identb)
        mask8 = const.tile([128, 512], bf16)
        nc.sync.dma_start(mask8, mask8_d)
        ones64 = const.tile([1, 64], bf16)
        nc.vector.memset(ones64, 1.0)

        p_xb = ctx.enter_context(tc.tile_pool(name="xb", bufs=2))
        p_qk = ctx.enter_context(tc.tile_pool(name="qk", bufs=1))
        p_vt = ctx.enter_context(tc.tile_pool(name="vt", bufs=2))
        p_attr = ctx.enter_context(tc.tile_pool(name="attr", bufs=1))
        p_attc = ctx.enter_context(tc.tile_pool(name="attc", bufs=1))
        p_strip = ctx.enter_context(tc.tile_pool(name="strip", bufs=2))
        p_z = ctx.enter_context(tc.tile_pool(name="z", bufs=2))
        p_fin = ctx.enter_context(tc.tile_pool(name="fin", bufs=2))
        p_big = ctx.enter_context(tc.tile_pool(name="psbig", bufs=2, space="PSUM"))
        p_sm = ctx.enter_context(tc.tile_pool(name="pssm", bufs=4, space="PSUM"))

        for b in range(n_batches):
            # phase 1: load bf16 x (4 slices), q/k matmuls
            xb = p_xb.tile([65, HW], bf16, tag="xb")
            for ci in range(4):
                sl = slice(ci * 4096, (ci + 1) * 4096)
                nc.sync.dma_start(xb[:, sl], x_d[b][:, sl])

            qk = p_qk.tile([8, 2 * HW], bf16, tag="qk")
            for ci in range(16):
                for half, off in ((0, 0), (1, HW)):
                    qp = p_big.tile([8, 1024], f32, tag="big")
                    for j in range(2):
                        n0 = ci * 1024 + j * 512
                        nc.tensor.matmul(
                            qp[:, j * 512 : (j + 1) * 512],
                            wqk[:, half * 8 : half * 8 + 8],
                            xb[:, n0 : n0 + 512],
                            start=True,
                            stop=True,
                        )
                    nc.vector.tensor_copy(
                        qk[0:8, off + ci * 1024 : off + ci * 1024 + 1024], qp
                    )

            xb3 = xb.rearrange("p (h w) -> p h w", w=W)
            q3 = qk[:, 0:HW].rearrange("p (h w) -> p h w", w=W)
            k3 = qk[:, HW : 2 * HW].rearrange("p (h w) -> p h w", w=W)

            # phases 2 & 3: col (axis=H) then row (axis=W) attention
            att_c = p_attc.tile([65, HW], bf16, tag="attc")
            att_r = p_attr.tile([65, HW], bf16, tag="attr")
            for is_row in (0, 1):
                att = att_r if is_row else att_c
                for g in range(NSTRIP):
                    vt = p_vt.tile([128, 65 * 8], bf16, tag="vt")
                    for half in range(2):
                        vtp = p_sm.tile([128, 260], f32, tag="sm")
                        for j in range(4):
                            ln = g * 8 + half * 4 + j
                            stat = (
                                xb[:, ln * 128 : (ln + 1) * 128]
                                if is_row
                                else xb3[:, :, ln]
                            )
                            nc.tensor.matmul(
                                vtp[:, j * 65 : (j + 1) * 65],
                                stat,
                                wvt,
                                start=True,
                                stop=True,
                            )
                        l0 = half * 4
                        nc.vector.tensor_copy(vt[:, l0 * 65 : (l0 + 4) * 65], vtp)
                    ep = p_big.tile([128, 1024], f32, tag="big")
                    for kk in range(8):
                        ln = g * 8 + kk
                        if is_row:
                            kap = qk[:, HW + ln * 128 : HW + (ln + 1) * 128]
                            qap = qk[:, ln * 128 : (ln + 1) * 128]
                        else:
                            kap = k3[:, :, ln]
                            qap = q3[:, :, ln]
                        nc.tensor.matmul(
                            ep[:, kk * 128 : (kk + 1) * 128],
                            kap,
                            qap,
                            start=True,
                            stop=True,
                        )
                    pstrip = p_strip.tile([128, 1024], bf16, tag="pstrip")
                    nc.scalar.activation(pstrip, ep, AF.Exp)
                    if not is_row:
                        nc.vector.tensor_tensor(
                            pstrip[:, 0:512], pstrip[:, 0:512], mask8, ALU.mult
                        )
                        nc.vector.tensor_tensor(
                            pstrip[:, 512:1024], pstrip[:, 512:1024], mask8, ALU.mult
                        )
                    for half in range(2):
                        avp = p_sm.tile([65, 512], f32, tag="sm")
                        for j in range(4):
                            ln8 = half * 4 + j
                            nc.tensor.matmul(
                                avp[:, j * 128 : (j + 1) * 128],
                                vt[:, ln8 * 65 : (ln8 + 1) * 65],
                                pstrip[:, ln8 * 128 : (ln8 + 1) * 128],
                                start=True,
                                stop=True,
                            )
                        l0 = g * 8 + half * 4
                        ev = att[:, l0 * 128 : (l0 + 4) * 128]
                        if is_row:
                            nc.scalar.activation(ev, avp, AF.Copy)
                        else:
                            nc.vector.tensor_copy(ev, avp)

            # phase 4: z = sW + sH^T, rz = 1/z
            zw = p_z.tile([128, 128], bf16, tag="zw")
            nc.sync.dma_start(zw, att_r[64:65, :])
            zh = p_z.tile([128, 128], bf16, tag="zh")
            nc.sync.dma_start(zh, att_c[64:65, :])
            ztp = p_sm.tile([128, 128], bf16, tag="sm")
            nc.tensor.transpose(ztp, zh, identb)
            zf = p_z.tile([128, 128], f32, tag="zf", bufs=1)
            nc.vector.tensor_tensor(zf, zw, ztp, ALU.add)
            nc.vector.reciprocal(zf, zf)
            rzb = p_z.tile([128, 128], bf16, tag="rzb", bufs=1)
            nc.vector.tensor_copy(rzb, zf)

            # phase 5: out = (att_r + att_c) * rz + x
            arv = att_r[0:64, :].rearrange("p (h w) -> p h w", w=W)
            acv = att_c[0:64, :].rearrange("p (w h) -> p h w", h=H)
            for ci in range(HW // CHUNK):
                h0 = ci * (CHUNK // W)
                rb = p_sm.tile([64, CHUNK], f32, tag="sm")
                for j in range(CHUNK // W):
                    sel64 = identb[:, h0 + j : h0 + j + 1].broadcast_to((128, 64))
                    nc.tensor.matmul(
                        rb[:, j * 128 : (j + 1) * 128],
                        sel64,
                        rzb,
                        start=True,
                        stop=True,
                    )
                t = p_fin.tile([64, CHUNK], bf16, tag="t")
                t3 = t.rearrange("p (h w) -> p h w", w=W)
                nc.vector.tensor_tensor(
                    t3, arv[:, h0 : h0 + 4, :], acv[:, h0 : h0 + 4, :], ALU.add
                )
                nc.vector.tensor_tensor(t, t, rb, ALU.mult)
                o = p_fin.tile([64, CHUNK], f32, tag="o")
                sl = slice(ci * CHUNK, (ci + 1) * CHUNK)
                nc.vector.tensor_tensor(o, t, xb[0:64, sl], ALU.add)
                nc.sync.dma_start(out_d[b][:, sl], o)

    nc.compile()
    return nc


class _Runner:
    """Builds the Bass program once and keeps a cached jitted executable."""

    def __init__(self):
        import jax
        from jax.sharding import Mesh, PartitionSpec
        from jax.experimental.shard_map import shard_map
        from concourse import bass2jax
        import concourse.mybir as mybir

        bass2jax.install_neuronx_cc_hook()
        self.nc = nc = _bass_build(BL)

        partition_name = (
            nc.partition_id_tensor.name if nc.partition_id_tensor else None
        )
        in_names, out_names, out_avals = [], [], []
        for alloc in nc.m.functions[0].allocations:
            if not isinstance(alloc, mybir.MemoryLocationSet):
                continue
            name = alloc.memorylocations[0].name
            if alloc.kind == "ExternalInput":
                if name != partition_name:
                    in_names.append(name)
            elif alloc.kind == "ExternalOutput":
                out_names.append(name)
                out_avals.append(
                    jax.core.ShapedArray(
                        tuple(alloc.tensor_shape), mybir.dt.np(alloc.dtype)
                    )
                )
        self.in_names = list(in_names)
        self.out_names = out_names
        self.out_avals = out_avals
        n_params = len(in_names)
        self.n_params = n_params
        all_names = in_names + out_names
        if partition_name is not None:
            all_names = all_names + [partition_name]

        def _body(*args):
            operands = list(args)
            if partition_name is not None:
                operands.append(bass2jax.partition_id_tensor())
            outs = bass2jax._bass_exec_p.bind(
                *operands,
                out_avals=tuple(out_avals),
                in_names=tuple(all_names),
                out_names=tuple(out_names),
                lowering_input_output_aliases=(),
                sim_require_finite=True,
                sim_require_nnan=True,
                nc=nc,
            )
            return tuple(outs)

        devices = jax.devices()[:N_CORES]
        mesh = Mesh(np.asarray(devices), ("core",))
        n_outs = len(out_names)
        in_specs = (PartitionSpec("core"),) * (n_params + n_outs)
        out_specs = (PartitionSpec("core"),) * n_outs
        self.fn = jax.jit(
            shard_map(
                _body, mesh=mesh, in_specs=in_specs, out_specs=out_specs,
                check_rep=False,
            ),
            donate_argnums=tuple(range(n_params, n_params + n_outs)),
            keep_unused=True,
        )

    def run(self, per_core_inputs):
        """per_core_inputs: dict name -> np array [N_CORES*dim0, ...]."""
        args = [per_core_inputs[n] for n in self.in_names]
        zeros = [
            np.zeros((N_CORES * a.shape[0], *a.shape[1:]), a.dtype)
            for a in self.out_avals
        ]
        outs = self.fn(*args, *zeros)
        return np.asarray(outs[0])


_runner = None
_const_cache = {}


def _get_runner():
    global _runner
    if _runner is None:
        _runner = _Runner()
    return _runner


def _consts(Wq, bq, Wk, bk, Wv, bv, gamma):
    key = float(np.asarray(gamma).reshape(-1)[0])
    if key in _const_cache:
        return _const_cache[key]
    g = key
    wqk = np.zeros((65, 16), np.float32)
    wqk[0:64, 0:8] = np.asarray(Wq, np.float32).T
    wqk[64, 0:8] = np.asarray(bq, np.float32)
    wqk[0:64, 8:16] = np.asarray(Wk, np.float32).T
    wqk[64, 8:16] = np.asarray(bk, np.float32)

    wvt = np.zeros((65, 65), np.float32)
    wvt[0:64, 0:64] = g * np.asarray(Wv, np.float32).T
    wvt[64, 0:64] = g * np.asarray(bv, np.float32)
    wvt[64, 64] = 1.0  # selects the x ones-row -> vT col 64 == 1

    mask8 = np.tile(1.0 - np.eye(128, dtype=np.float32), (1, 4))
    con = {
        "wqk": np.tile(wqk.astype(_bf16), (N_CORES, 1)),
        "wvt": np.tile(wvt.astype(_bf16), (N_CORES, 1)),
        "mask8": np.tile(mask8.astype(_bf16), (N_CORES, 1)),
        "identb": np.tile(np.eye(128, dtype=np.float32).astype(_bf16), (N_CORES, 1)),
    }
    _const_cache[key] = con
    return con


def kernel(x, Wq, bq, Wk, bk, Wv, bv, gamma):
    x = np.ascontiguousarray(np.asarray(x, np.float32))
    r = _get_runner()

    # [16, 64, H, W] f32 -> [16, 65, HW] bf16 with a ones row at channel 64
    xb = np.empty((B, C + 1, HW), _bf16)
    xb[:, 0:64, :] = x.reshape(B, C, HW).astype(_bf16)
    xb[:, 64, :] = _bf16(1.0)
    # concat per-core along axis 0: core i gets batches [2i, 2i+2)
    inputs = {"x": xb.reshape(N_CORES * BL, C + 1, HW)}
    inputs.update(_consts(Wq, bq, Wk, bk, Wv, bv, gamma))

    out = r.run(inputs)  # [N_CORES*BL, C, HW] f32
    return out.reshape(B, C, H, W).astype(np.float32, copy=False)
